# revision 13
# baseline (speedup 1.0000x reference)
"""Trainium2 Bass kernel for nn_CorrNodeEncoding1G.

out = x + gate * LN(z @ proj_w + proj_b), z = per-node stats+RBF features of C.

Sharding: data-parallel over nodes N across 8 cores (12500 nodes each).
Per-core pipeline over 128-node tiles:
  - DVE: bn_stats/bn_aggr (mean+var of C in one pass), two tensor_tensor_reduce
    ops (pairwise-halves max / min + reduction, 2-read-port floor), RBF args,
    final residual add.
  - PE:  z transpose, z @ W_aug, and LN mean/var as linear/quadratic forms
    (mu = z.m, var = z^T H z with H = W W^T/D - m m^T, host-precomputed).
  - ACT: sqrt/exp/LN scale-bias, PSUM->SBUF moves.
"""

import sys

sys.path.insert(0, "/opt/trn_rl_repo")

import numpy as np

N, S, D = 100000, 1024, 512
N_RBF = 8
LN_EPS = 1e-5
M_CORES = 8
NPC = N // M_CORES          # 12500 nodes per core
P = 128
N_FULL = NPC // P           # 97 full tiles
TAIL = NPC - N_FULL * P     # 84
F = 21                      # 20 features + constant-1 (carries proj_b)

TRACE = False               # set by test.py for profiling
LAST_EXEC_NS = None

_compiled = {}


def _build(fast: bool, k: float, cb: float):
    """Build + finalize the SPMD bass program.

    fast: gamma and beta are constant rows; fold gate*gamma into the
    per-node LN scale and gate*beta into the LN bias.  Slow path applies
    replicated gamma/beta rows with extra elementwise ops.
    """
    import concourse.bass as bass
    import concourse.bacc as bacc
    import concourse.mybir as mybir
    import concourse.tile as tile

    f32 = mybir.dt.float32
    Alu = mybir.AluOpType
    Act = mybir.ActivationFunctionType
    Axis = mybir.AxisListType

    nc = bacc.Bacc("TRN2", target_bir_lowering=False, debug=False)

    x_d = nc.declare_dram_parameter("x", [NPC, D], f32, isOutput=False)
    C_d = nc.declare_dram_parameter("C", [NPC, S], f32, isOutput=False)
    W_d = nc.declare_dram_parameter("W_aug", [F, D], f32, isOutput=False)
    H_d = nc.declare_dram_parameter("H", [F, F], f32, isOutput=False)
    mneg_d = nc.declare_dram_parameter("mneg", [F, 1], f32, isOutput=False)
    ones_d = nc.declare_dram_parameter("onesF", [F, 1], f32, isOutput=False)
    invw_d = nc.declare_dram_parameter("invw_rep", [P, N_RBF], f32, isOutput=False)
    negcw_d = nc.declare_dram_parameter("negcw_rep", [P, N_RBF], f32, isOutput=False)
    ident_d = nc.declare_dram_parameter("ident", [P, P], f32, isOutput=False)
    if not fast:
        gam_d = nc.declare_dram_parameter("gamma_rep", [P, D], f32, isOutput=False)
        cb_d = nc.declare_dram_parameter("cb_rep", [P, D], f32, isOutput=False)
    o_d = nc.declare_dram_parameter("out", [NPC, D], f32, isOutput=True)

    inv_k2 = 1.0 / (k * k) if fast else 1.0
    eps_k2 = LN_EPS * inv_k2

    with tile.TileContext(nc) as tc:
        with (
            tc.tile_pool(name="const", bufs=1) as cpool,
            tc.tile_pool(name="io", bufs=3) as io,
            tc.tile_pool(name="work", bufs=3) as wk,
            tc.tile_pool(name="small", bufs=4) as sm,
            tc.tile_pool(name="psA", bufs=2, space=bass.MemorySpace.PSUM) as psA,
            tc.tile_pool(name="psB", bufs=1, space=bass.MemorySpace.PSUM) as psB,
            tc.tile_pool(name="psC", bufs=1, space=bass.MemorySpace.PSUM) as psC,
        ):
            W_s = cpool.tile([F, D], f32)
            H_s = cpool.tile([F, F], f32)
            mneg_s = cpool.tile([F, 1], f32)
            ones_s = cpool.tile([F, 1], f32)
            invw_s = cpool.tile([P, N_RBF], f32)
            negcw_s = cpool.tile([P, N_RBF], f32)
            ident_s = cpool.tile([P, P], f32)
            epsb_s = cpool.tile([P, 1], f32)
            nc.gpsimd.memset(epsb_s[:], eps_k2)
            nc.sync.dma_start(W_s[:], W_d[:])
            nc.sync.dma_start(H_s[:], H_d[:])
            nc.sync.dma_start(mneg_s[:], mneg_d[:])
            nc.sync.dma_start(ones_s[:], ones_d[:])
            nc.sync.dma_start(invw_s[:], invw_d[:])
            nc.sync.dma_start(negcw_s[:], negcw_d[:])
            nc.sync.dma_start(ident_s[:], ident_d[:])
            if not fast:
                gam_s = cpool.tile([P, D], f32)
                cb_s = cpool.tile([P, D], f32)
                nc.sync.dma_start(gam_s[:], gam_d[:])
                nc.sync.dma_start(cb_s[:], cb_d[:])

            n_tiles = N_FULL + (1 if TAIL else 0)
            for i in range(n_tiles):
                p = P if i < N_FULL else TAIL
                r0 = i * P

                Ct = io.tile([P, S], f32, tag="Ct")
                xt = io.tile([P, D], f32, tag="xt")
                nc.sync.dma_start(Ct[:p, :], C_d[r0:r0 + p, :])
                nc.sync.dma_start(xt[:p, :], x_d[r0:r0 + p, :])

                z = sm.tile([P, F], f32, tag="z")
                bst = sm.tile([P, 2, 6], f32, tag="bst")

                # C stats: mean/var via bn_stats; max/min via tensor_reduce.
                nc.vector.bn_stats(bst[:p, 0, :], Ct[:p, 0:S // 2])
                nc.vector.bn_stats(bst[:p, 1, :], Ct[:p, S // 2:S])
                nc.vector.bn_aggr(z[:p, 0:2], bst[:p])
                nc.vector.tensor_reduce(
                    z[:p, 2:3], Ct[:p, :], axis=Axis.X, op=Alu.max
                )
                nc.vector.tensor_reduce(
                    z[:p, 3:4], Ct[:p, :], axis=Axis.X, op=Alu.min
                )
                # std = sqrt(var), in place
                nc.scalar.sqrt(z[:p, 1:2], z[:p, 1:2])
                # constant-1 feature (carries proj_b through the matmul)
                nc.gpsimd.memset(z[:p, 20:21], 1.0)
                # RBF args t = v*invw - c*invw for v in (mean, max)
                nc.vector.scalar_tensor_tensor(
                    out=z[:p, 4:12], in0=invw_s[:p], scalar=z[:p, 0:1],
                    in1=negcw_s[:p], op0=Alu.mult, op1=Alu.add,
                )
                nc.vector.scalar_tensor_tensor(
                    out=z[:p, 12:20], in0=invw_s[:p], scalar=z[:p, 2:3],
                    in1=negcw_s[:p], op0=Alu.mult, op1=Alu.add,
                )
                # rbf = exp(-0.5 t^2)
                nc.scalar.activation(z[:p, 4:20], z[:p, 4:20], Act.Square)
                nc.scalar.activation(z[:p, 4:20], z[:p, 4:20], Act.Exp, scale=-0.5)

                # z^T  (feat-major for the PE matmuls)
                zTp = psB.tile([F, P], f32, tag="zTp")
                nc.tensor.transpose(zTp[:, :p], z[:p, :], ident_s[:p, :p])
                zTs = sm.tile([F, P], f32, tag="zTs")
                nc.scalar.copy(zTs[:, :p], zTp[:, :p])

                # pe = z @ W_aug ; mu_neg = z @ (-m) ; var = z^T H z
                pe = psA.tile([P, D], f32, tag="pe")
                nc.tensor.matmul(pe[:p, :], zTs[:, :p], W_s[:], start=True, stop=True)
                muv = psC.tile([P, 1], f32, tag="muv")
                nc.tensor.matmul(muv[:p, :], zTs[:, :p], mneg_s[:], start=True, stop=True)
                u = psB.tile([F, P], f32, tag="u")
                nc.tensor.matmul(u[:, :p], H_s[:], zTs[:, :p], start=True, stop=True)
                us = sm.tile([F, P], f32, tag="us")
                nc.scalar.copy(us[:, :p], u[:, :p])
                zu = sm.tile([F, P], f32, tag="zu")
                nc.gpsimd.tensor_tensor(zu[:, :p], zTs[:, :p], us[:, :p], Alu.mult)
                varp = psC.tile([P, 1], f32, tag="varp")
                nc.tensor.matmul(varp[:p, :], zu[:, :p], ones_s[:], start=True, stop=True)

                # astar = k / sqrt(var + eps); bmu = -astar*mu + cb
                sq = sm.tile([P, 1], f32, tag="sq")
                nc.scalar.activation(
                    sq[:p], varp[:p], Act.Sqrt, scale=inv_k2, bias=epsb_s[:p]
                )
                astar = sm.tile([P, 1], f32, tag="astar")
                nc.vector.reciprocal(astar[:p], sq[:p])
                bmu = sm.tile([P, 1], f32, tag="bmu")
                nc.scalar.activation(
                    bmu[:p], muv[:p], Act.Copy, scale=astar[:p],
                    bias=cb if fast else 0.0,
                )
                # t = astar*pe + bmu  (= gate*gamma*(pe-mu)/sqrt(var+eps) + gate*beta)
                t = wk.tile([P, D], f32, tag="t")
                nc.scalar.activation(
                    t[:p], pe[:p], Act.Identity, scale=astar[:p], bias=bmu[:p]
                )
                ot = io.tile([P, D], f32, tag="ot")
                if fast:
                    nc.gpsimd.tensor_tensor(ot[:p], t[:p], xt[:p, :], Alu.add)
                else:
                    t2 = wk.tile([P, D], f32, tag="t2")
                    nc.vector.tensor_tensor(t2[:p], t[:p], gam_s[:p], Alu.mult)
                    nc.vector.tensor_tensor(t2[:p], t2[:p], cb_s[:p], Alu.add)
                    nc.vector.tensor_tensor(ot[:p], t2[:p], xt[:p, :], Alu.add)
                nc.sync.dma_start(o_d[r0:r0 + p, :], ot[:p])

    nc.finalize()
    return nc


def kernel(**inputs):
    global LAST_EXEC_NS
    from concourse.bass_utils import run_bass_kernel_spmd

    x = np.ascontiguousarray(np.asarray(inputs["x"], dtype=np.float32))
    C = np.asarray(inputs["C"], dtype=np.float32)
    mask = np.asarray(inputs["mask_nodes"])
    proj_w = np.asarray(inputs["proj_w"], dtype=np.float64)
    proj_b = np.asarray(inputs["proj_b"], dtype=np.float64)
    gamma = np.asarray(inputs["ln_gamma"], dtype=np.float64)
    beta = np.asarray(inputs["ln_beta"], dtype=np.float64)
    gate = float(np.asarray(inputs["gate"]))
    centers = np.asarray(inputs["rbf_centers"], dtype=np.float64)
    widths = np.asarray(inputs["rbf_widths"], dtype=np.float64)

    # reference clips C to [0,1]; on in-range data that's the identity.
    if C.min() < 0.0 or C.max() > 1.0:
        C = np.clip(C, 0.0, 1.0)
    C = np.ascontiguousarray(C)

    # device feature order: [mean, std, max, min, rbf(mean), rbf(max), 1]
    perm = [0, 3, 1, 2] + list(range(4, 20))
    W_aug = np.vstack([proj_w[perm], proj_b[None, :]])        # [21, 512] f64
    m = W_aug.mean(axis=1)
    H = W_aug @ W_aug.T / D - np.outer(m, m)

    g0, b0 = float(gamma.flat[0]), float(beta.flat[0])
    fast = bool(np.all(gamma == g0) and np.all(beta == b0) and gate * g0 != 0.0)
    k = gate * g0 if fast else 1.0
    cb = gate * b0 if fast else 0.0

    invw = 1.0 / (widths + 1e-6)
    invw_rep = np.broadcast_to(invw.astype(np.float32), (P, N_RBF)).copy()
    negcw_rep = np.broadcast_to(
        (-centers * invw).astype(np.float32), (P, N_RBF)
    ).copy()

    consts = {
        "W_aug": W_aug.astype(np.float32),
        "H": H.astype(np.float32),
        "mneg": (-m).astype(np.float32).reshape(F, 1),
        "onesF": np.ones((F, 1), np.float32),
        "invw_rep": invw_rep,
        "negcw_rep": negcw_rep,
        "ident": np.eye(P, dtype=np.float32),
    }
    if not fast:
        consts["gamma_rep"] = np.broadcast_to(
            (gate * gamma).astype(np.float32), (P, D)
        ).copy()
        consts["cb_rep"] = np.broadcast_to(
            (gate * beta).astype(np.float32), (P, D)
        ).copy()

    key = (fast, k, cb)
    if key not in _compiled:
        _compiled[key] = _build(fast, k, cb)
    nc = _compiled[key]

    in_maps = []
    for c in range(M_CORES):
        s0 = c * NPC
        in_maps.append(
            {"x": x[s0:s0 + NPC], "C": C[s0:s0 + NPC], **consts}
        )

    res = run_bass_kernel_spmd(nc, in_maps, list(range(M_CORES)), trace=TRACE)
    LAST_EXEC_NS = res.exec_time_ns
    out = np.concatenate([res.results[c]["out"] for c in range(M_CORES)], axis=0)

    if mask.any():
        out = out.copy()
        out[mask] = x[mask]
    return out


# revision 19
# speedup vs baseline: 1.1239x; 1.1239x over previous
"""Trainium2 Bass kernel for nn_CorrNodeEncoding1G.

out = x + gate * LN(z @ proj_w + proj_b), z = per-node stats+RBF features of C.

Sharding: data-parallel over nodes N across 8 cores (12500 nodes each).
Per-core pipeline over 128-node tiles, processed in groups of G=8 tiles so
per-node scalar math batches into one instruction per group:
  - DVE: bn_stats/bn_aggr (mean+var of C), tensor_reduce max/min, RBF args.
  - PE (bf16): z transpose, z @ W_aug, LN mean/var as linear/quadratic forms
    (mu = z.m, var = z^T H z, H = W W^T/D - m m^T, host-precomputed).
  - ACT: batched Square/Exp/Sqrt (sqrt batched per group to avoid act-table
    thrash: no table set holds both sqrt and exp), LN scale-bias.
  - GpSimd: zu elementwise, residual add.
"""

import sys

sys.path.insert(0, "/opt/trn_rl_repo")

import numpy as np

N, S, D = 100000, 1024, 512
N_RBF = 8
LN_EPS = 1e-5
M_CORES = 8
NPC = N // M_CORES          # 12500 nodes per core
P = 128
N_FULL = NPC // P           # 97 full tiles
TAIL = NPC - N_FULL * P     # 84
F = 21                      # 20 features + constant-1 (carries proj_b)
G = 4                       # tiles per batching group (pe PSUM banks)

TRACE = False               # set by test.py for profiling
LAST_EXEC_NS = None

_compiled = {}


def _build(fast: bool, k: float, cb: float):
    import concourse.bass as bass
    import concourse.bacc as bacc
    import concourse.mybir as mybir
    import concourse.tile as tile

    f32 = mybir.dt.float32
    bf16 = mybir.dt.bfloat16
    Alu = mybir.AluOpType
    Act = mybir.ActivationFunctionType
    Axis = mybir.AxisListType

    nc = bacc.Bacc("TRN2", target_bir_lowering=False, debug=False)

    x_d = nc.declare_dram_parameter("x", [NPC, D], f32, isOutput=False)
    C_d = nc.declare_dram_parameter("C", [NPC, S], f32, isOutput=False)
    W_d = nc.declare_dram_parameter("W_aug", [F, D], bf16, isOutput=False)
    H_d = nc.declare_dram_parameter("H", [F, F], bf16, isOutput=False)
    mneg_d = nc.declare_dram_parameter("mneg", [F, 1], bf16, isOutput=False)
    ones_d = nc.declare_dram_parameter("onesF", [F, 1], bf16, isOutput=False)
    invw_d = nc.declare_dram_parameter("invw_rep", [P, N_RBF], f32, isOutput=False)
    negcw_d = nc.declare_dram_parameter("negcw_rep", [P, N_RBF], f32, isOutput=False)
    ident_d = nc.declare_dram_parameter("ident", [P, P], f32, isOutput=False)
    if not fast:
        gam_d = nc.declare_dram_parameter("gamma_rep", [P, D], f32, isOutput=False)
        cb_d = nc.declare_dram_parameter("cb_rep", [P, D], f32, isOutput=False)
    o_d = nc.declare_dram_parameter("out", [NPC, D], f32, isOutput=True)

    inv_k2 = 1.0 / (k * k) if fast else 1.0
    eps_k2 = LN_EPS * inv_k2

    n_tiles = N_FULL + (1 if TAIL else 0)
    groups = [list(range(g, min(g + G, n_tiles))) for g in range(0, n_tiles, G)]

    with tile.TileContext(nc) as tc:
        with (
            tc.tile_pool(name="const", bufs=1) as cpool,
            tc.tile_pool(name="io", bufs=4) as io,
            tc.tile_pool(name="work", bufs=4) as wk,
            tc.tile_pool(name="small", bufs=3) as sm,
            tc.tile_pool(name="psA", bufs=G, space=bass.MemorySpace.PSUM) as psA,
            tc.tile_pool(name="psB", bufs=1, space=bass.MemorySpace.PSUM) as psB,
            tc.tile_pool(name="psC", bufs=1, space=bass.MemorySpace.PSUM) as psC,
        ):
            W_s = cpool.tile([F, D], bf16)
            H_s = cpool.tile([F, F], bf16)
            mneg_s = cpool.tile([F, 1], bf16)
            ones_s = cpool.tile([F, 1], bf16)
            invw_s = cpool.tile([P, N_RBF], f32)
            negcw_s = cpool.tile([P, N_RBF], f32)
            ident_s = cpool.tile([P, P], f32)
            nc.sync.dma_start(W_s[:], W_d[:])
            nc.sync.dma_start(H_s[:], H_d[:])
            nc.sync.dma_start(mneg_s[:], mneg_d[:])
            nc.sync.dma_start(ones_s[:], ones_d[:])
            nc.sync.dma_start(invw_s[:], invw_d[:])
            nc.sync.dma_start(negcw_s[:], negcw_d[:])
            nc.sync.dma_start(ident_s[:], ident_d[:])
            if not fast:
                gam_s = cpool.tile([P, D], f32)
                cb_s = cpool.tile([P, D], f32)
                nc.sync.dma_start(gam_s[:], gam_d[:])
                nc.sync.dma_start(cb_s[:], cb_d[:])

            for grp in groups:
                g = len(grp)
                # group-shared buffers
                zg = sm.tile([P, G, F], f32, tag="zg")
                bstg = sm.tile([P, G, 2, 6], f32, tag="bstg")
                muvb = psC.tile([P, G], f32, tag="muvb")     # -mu per sub-tile
                varb = psC.tile([P, G], f32, tag="varb")     # var per sub-tile
                Cts, xts, zTss, pes = [], [], [], []

                # ones feature column for the whole group
                nc.gpsimd.memset(zg[:, :, 20:21], 1.0)
                if g < G:
                    # partial tail group: zero unused columns so batched
                    # Sqrt/Square/Exp over the full [P, G, *] read valid data
                    nc.gpsimd.memset(zg[:, g:, 0:20], 0.0)

                for gi, i in enumerate(grp):
                    p = P if i < N_FULL else TAIL
                    r0 = i * P
                    Ct = io.tile([P, S], f32, tag="Ct")
                    xt = io.tile([P, D], f32, tag="xt")
                    nc.sync.dma_start(Ct[:p, :], C_d[r0:r0 + p, :])
                    nc.sync.dma_start(xt[:p, :], x_d[r0:r0 + p, :])
                    Cts.append(Ct)
                    xts.append(xt)

                    z = zg[:, gi, :]
                    if p < P:
                        p32 = (p // 32) * 32
                        nc.gpsimd.memset(zg[p32:, gi, 0:20], 0.0)
                    # C stats
                    nc.vector.bn_stats(bstg[:p, gi, 0, :], Ct[:p, 0:S // 2])
                    nc.vector.bn_stats(bstg[:p, gi, 1, :], Ct[:p, S // 2:S])
                    nc.vector.bn_aggr(z[:p, 0:2], bstg[:p, gi])
                    nc.vector.tensor_reduce(
                        z[:p, 2:3], Ct[:p, :], axis=Axis.X, op=Alu.max
                    )
                    nc.vector.tensor_reduce(
                        z[:p, 3:4], Ct[:p, :], axis=Axis.X, op=Alu.min
                    )
                    # RBF args t = v*invw - c*invw for v in (mean, max)
                    nc.vector.scalar_tensor_tensor(
                        out=z[:p, 4:12], in0=invw_s[:p], scalar=z[:p, 0:1],
                        in1=negcw_s[:p], op0=Alu.mult, op1=Alu.add,
                    )
                    nc.vector.scalar_tensor_tensor(
                        out=z[:p, 12:20], in0=invw_s[:p], scalar=z[:p, 2:3],
                        in1=negcw_s[:p], op0=Alu.mult, op1=Alu.add,
                    )

                # batched: std = sqrt(var); rbf = exp(-0.5 t^2)
                nc.scalar.sqrt(zg[:, :, 1:2], zg[:, :, 1:2])
                nc.scalar.activation(zg[:, :, 4:20], zg[:, :, 4:20], Act.Square)
                nc.scalar.activation(
                    zg[:, :, 4:20], zg[:, :, 4:20], Act.Exp, scale=-0.5
                )

                for gi, i in enumerate(grp):
                    p = P if i < N_FULL else TAIL
                    # z^T (bf16, feat-major) for the PE matmuls
                    zTp = psB.tile([F, P], f32, tag="zTp")
                    nc.tensor.transpose(
                        zTp[:, :p], zg[:p, gi, :], ident_s[:p, :p]
                    )
                    zTs = sm.tile([F, P], bf16, tag="zTs")
                    nc.scalar.copy(zTs[:, :p], zTp[:, :p])
                    zTss.append(zTs)

                    if p < P:
                        p32 = (p // 32) * 32
                        nc.vector.memset(muvb[p32:, gi:gi + 1], 0.0)
                        nc.vector.memset(varb[p32:, gi:gi + 1], 1.0)
                    pe = psA.tile([P, D], f32, tag="pe")
                    nc.tensor.matmul(
                        pe[:p, :], zTs[:, :p], W_s[:], start=True, stop=True
                    )
                    pes.append(pe)
                    nc.tensor.matmul(
                        muvb[:p, gi:gi + 1], zTs[:, :p], mneg_s[:],
                        start=True, stop=True,
                    )
                    u = psB.tile([F, P], f32, tag="u")
                    nc.tensor.matmul(
                        u[:, :p], H_s[:], zTs[:, :p], start=True, stop=True
                    )
                    us = sm.tile([F, P], bf16, tag="us")
                    nc.scalar.copy(us[:, :p], u[:, :p])
                    zu = sm.tile([F, P], bf16, tag="zu")
                    nc.gpsimd.tensor_tensor(
                        zu[:, :p], zTs[:, :p], us[:, :p], Alu.mult
                    )
                    nc.tensor.matmul(
                        varb[:p, gi:gi + 1], zu[:, :p], ones_s[:],
                        start=True, stop=True,
                    )

                # batched LN scalars: astar = k/sqrt(var+eps), bmu = -astar*mu+cb
                sqv = sm.tile([P, G], f32, tag="sqv")
                nc.scalar.sqrt(sqv[:, :g], varb[:, :g])
                astar = sm.tile([P, G], f32, tag="astar")
                nc.vector.reciprocal(astar[:, :g], sqv[:, :g])
                bmu = sm.tile([P, G], f32, tag="bmu")
                nc.vector.tensor_tensor(
                    bmu[:, :g], muvb[:, :g], astar[:, :g], Alu.mult
                )
                if fast and cb != 0.0:
                    nc.vector.tensor_scalar(
                        bmu[:, :g], bmu[:, :g], cb, None, Alu.add
                    )

                for gi, i in enumerate(grp):
                    p = P if i < N_FULL else TAIL
                    r0 = i * P
                    t = wk.tile([P, D], f32, tag="t")
                    nc.scalar.activation(
                        t[:p], pes[gi][:p], Act.Identity,
                        scale=astar[:p, gi:gi + 1], bias=bmu[:p, gi:gi + 1],
                    )
                    ot = io.tile([P, D], f32, tag="ot")
                    if fast:
                        nc.gpsimd.tensor_tensor(
                            ot[:p], t[:p], xts[gi][:p, :], Alu.add
                        )
                    else:
                        t2 = wk.tile([P, D], f32, tag="t2")
                        nc.vector.tensor_tensor(t2[:p], t[:p], gam_s[:p], Alu.mult)
                        nc.vector.tensor_tensor(t2[:p], t2[:p], cb_s[:p], Alu.add)
                        nc.gpsimd.tensor_tensor(
                            ot[:p], t2[:p], xts[gi][:p, :], Alu.add
                        )
                    nc.sync.dma_start(o_d[r0:r0 + p, :], ot[:p])

    nc.finalize()
    return nc


def kernel(**inputs):
    global LAST_EXEC_NS
    import ml_dtypes
    from concourse.bass_utils import run_bass_kernel_spmd

    bf = ml_dtypes.bfloat16
    x = np.ascontiguousarray(np.asarray(inputs["x"], dtype=np.float32))
    C = np.asarray(inputs["C"], dtype=np.float32)
    mask = np.asarray(inputs["mask_nodes"])
    proj_w = np.asarray(inputs["proj_w"], dtype=np.float64)
    proj_b = np.asarray(inputs["proj_b"], dtype=np.float64)
    gamma = np.asarray(inputs["ln_gamma"], dtype=np.float64)
    beta = np.asarray(inputs["ln_beta"], dtype=np.float64)
    gate = float(np.asarray(inputs["gate"]))
    centers = np.asarray(inputs["rbf_centers"], dtype=np.float64)
    widths = np.asarray(inputs["rbf_widths"], dtype=np.float64)

    # reference clips C to [0,1]; on in-range data that's the identity.
    if C.min() < 0.0 or C.max() > 1.0:
        C = np.clip(C, 0.0, 1.0)
    C = np.ascontiguousarray(C)

    # device feature order: [mean, std, max, min, rbf(mean), rbf(max), 1]
    perm = [0, 3, 1, 2] + list(range(4, 20))
    W_aug = np.vstack([proj_w[perm], proj_b[None, :]])        # [21, 512] f64
    m = W_aug.mean(axis=1)
    H = W_aug @ W_aug.T / D - np.outer(m, m)

    g0, b0 = float(gamma.flat[0]), float(beta.flat[0])
    fast = bool(np.all(gamma == g0) and np.all(beta == b0) and gate * g0 != 0.0)
    k = gate * g0 if fast else 1.0
    cb = gate * b0 if fast else 0.0
    inv_k2 = 1.0 / (k * k)

    invw = 1.0 / (widths + 1e-6)
    invw_rep = np.broadcast_to(invw.astype(np.float32), (P, N_RBF)).copy()
    negcw_rep = np.broadcast_to(
        (-centers * invw).astype(np.float32), (P, N_RBF)
    ).copy()

    H_dev = H * inv_k2
    H_dev[F - 1, F - 1] += LN_EPS * inv_k2
    consts = {
        "W_aug": W_aug.astype(bf),
        "H": H_dev.astype(bf),
        "mneg": (-m).astype(bf).reshape(F, 1),
        "onesF": np.ones((F, 1), bf),
        "invw_rep": invw_rep,
        "negcw_rep": negcw_rep,
        "ident": np.eye(P, dtype=np.float32),
    }
    if not fast:
        consts["gamma_rep"] = np.broadcast_to(
            (gate * gamma).astype(np.float32), (P, D)
        ).copy()
        consts["cb_rep"] = np.broadcast_to(
            (gate * beta).astype(np.float32), (P, D)
        ).copy()

    key = (fast, k, cb)
    if key not in _compiled:
        _compiled[key] = _build(fast, k, cb)
    nc = _compiled[key]

    in_maps = []
    for c in range(M_CORES):
        s0 = c * NPC
        in_maps.append(
            {"x": x[s0:s0 + NPC], "C": C[s0:s0 + NPC], **consts}
        )

    res = run_bass_kernel_spmd(nc, in_maps, list(range(M_CORES)), trace=TRACE)
    LAST_EXEC_NS = res.exec_time_ns
    out = np.concatenate([res.results[c]["out"] for c in range(M_CORES)], axis=0)

    if mask.any():
        out = out.copy()
        out[mask] = x[mask]
    return out
